# revision 4
# baseline (speedup 1.0000x reference)
"""GAT layer kernel for Trainium2, sharded across 8 NeuronCores.

Math: reference computes
    h = x @ W.T;  e_ij = (h @ a1)[i] + (h @ a2)[j];  mask by adj;
    softmax over j; out = attn @ h.
Because e_i is constant along the softmax axis it cancels, so with
w_j = exp(h_j . a2):
    out[i] = sum_j adj[i,j] * w_j * h[j] / sum_j adj[i,j] * w_j
which is a single (adj_f32 @ [w*h | w]) matmul plus a row division.
a1 is mathematically irrelevant.

Sharding: rows of adj (and of the output) are split across 8 cores;
h (8 MB) is small, so every core computes the full h redundantly
(cheaper than a collective). The host passes adj row-blocks
TRANSPOSED so the contracted index j lands on the SBUF partition
axis with DMA-friendly strides.

dtype strategy: matmuls run in float32r (e8m11, round-to-nearest-even,
fp32 accumulate in PSUM) which streams at bf16 rate for free-dim >= 256.
adj 0/1 values are exact in f32r; the cast int32 -> f32r happens inline
in the SWDGE DMA, so no compute engine touches the 33 MB adjacency
block. Measured end-to-end error vs the fp32 reference is a few 1e-4
relative to output scale.
"""

import sys

import numpy as np

for _p in ("/opt/trn_rl_repo",):
    try:
        import concourse.bass  # noqa: F401

        break
    except ImportError:
        if _p not in sys.path:
            sys.path.insert(0, _p)

import concourse.bass as bass
import concourse.mybir as mybir
import concourse.tile as tile
from concourse.bass_utils import run_bass_kernel_spmd

dt = mybir.dt
AF = mybir.ActivationFunctionType

N = 8192
D = 256
NCORES = 8
RB = N // NCORES  # 1024 output rows per core
W_FREE = 260  # 256 h cols + 1 w col + 3 pad (f32r free dim must be even)
NJ = N // 128  # 64 j-chunks
NR = RB // 128  # 8 r-chunks per core
NI = D // 128  # 2 contraction chunks for h
SWDGE_CAST_ADJ = True  # int32 -> f32r inline in DMA

# ---------------------------------------------------------------------------
# walrus in this container accepts at most ONE sync-wait command on several
# instruction structs (Drain, 4-byte self-loading Matmult, ...) while the
# newer Tile scheduler emits more. Split the extras into single-wait
# EventSemaphore prefixes on the same engine (identical semantics).
_ev_counter = [0]


def _legalize_multiwait(nc, max_keep=1):
    for f in nc.m.functions:
        for bb in f.blocks:
            il = bb.instructions
            idx = 0
            while idx < len(il):
                inst = il[idx]
                si = inst.sync_info
                if si is not None and si.on_wait and len(si.on_wait) > max_keep:
                    waits = list(si.on_wait)
                    keep = waits[len(waits) - max_keep :] if max_keep else []
                    extra = waits[: len(waits) - max_keep] if max_keep else waits
                    si.on_wait = keep
                    for w in extra:
                        _ev_counter[0] += 1
                        ev = mybir.InstEventSemaphore(
                            name=f"lgw_{_ev_counter[0]}", ins=[], outs=[]
                        )
                        ev.engine = inst.engine
                        ev.sync_info = mybir.SyncInfo(on_wait=[w], on_update=[])
                        il.insert(idx, ev)
                        idx += 1
                idx += 1


# ---------------------------------------------------------------------------


def _build_program():
    nc = bass.Bass("TRN2", debug=False)

    xT = nc.dram_tensor("xT", [D, N], dt.float32, kind="ExternalInput").ap()
    WTe = nc.dram_tensor("WTe", [D, W_FREE], dt.float32, kind="ExternalInput").ap()
    adjT = nc.dram_tensor(
        "adjT", [N, RB], dt.int32, kind="ExternalInput"
    ).ap()  # adj rows of this core, transposed: [j, r]
    out = nc.dram_tensor("out", [RB, D], dt.float32, kind="ExternalOutput").ap()

    XCH = 2048  # x streamed in [128, XCH] chunks (1 MB per DMA)
    NXB = N // XCH  # 8 chunks per i-half
    JB = 4  # j-chunks per adjT DMA (2 MB per transfer)

    with tile.TileContext(nc) as tc:
        with (
            tc.tile_pool(name="xr", bufs=1) as xr_pool,
            tc.tile_pool(name="wte", bufs=1) as wte_pool,
            tc.tile_pool(name="hw", bufs=1) as hw_pool,
            tc.tile_pool(name="wcol", bufs=4) as w_pool,
            tc.tile_pool(name="adjr", bufs=3) as adj_pool,
            tc.tile_pool(name="outs", bufs=4) as out_pool,
            tc.tile_pool(name="rec", bufs=4) as rec_pool,
        ):
            # ---- load x^T and W^T_ext, casting f32 -> f32r in the DMA.
            # SWDGE drains its single queue in issue order, so these chunked
            # transfers all land ahead of the big adjacency stream and phase 1
            # can start within a few us.
            wte = []
            for ic in range(NI):
                t = wte_pool.tile([128, W_FREE], dt.float32r, name=f"wte{ic}")
                nc.gpsimd.dma_start(t, WTe[ic * 128 : (ic + 1) * 128, :])
                wte.append(t)
            xr = [[None] * NXB for _ in range(NI)]
            for b in range(NXB):
                for ic in range(NI):
                    t = xr_pool.tile([128, XCH], dt.float32r, name=f"xr{ic}_{b}")
                    nc.gpsimd.dma_start(
                        t, xT[ic * 128 : (ic + 1) * 128, b * XCH : (b + 1) * XCH]
                    )
                    xr[ic][b] = t

            hw = [
                hw_pool.tile([128, W_FREE], dt.float32r, name=f"hw{j}")
                for j in range(NJ)
            ]

            # ---- phase 1: h plus e in one matmul; build hw = [w*h | w] ----
            NCPB = XCH // 128  # n-chunks per x chunk
            with tc.tile_pool(name="ph", bufs=4, space="PSUM") as ph_pool:
                for ncc in range(NJ):
                    b, sl = ncc // NCPB, bass.ts(ncc % NCPB, 128)
                    ph = ph_pool.tile([128, W_FREE], dt.float32, name="ph", tag="ph")
                    nc.tensor.matmul(
                        ph, xr[0][b][:, sl], wte[0], start=True, stop=False
                    )
                    nc.tensor.matmul(
                        ph, xr[1][b][:, sl], wte[1], start=False, stop=True
                    )
                    w = w_pool.tile([128, 1], dt.float32, name="w", tag="w")
                    nc.scalar.activation(w, ph[:, 256:257], AF.Exp)
                    if ncc % 2 == 0:
                        nc.vector.tensor_scalar_mul(
                            hw[ncc][:, 0:256], ph[:, 0:256], w
                        )
                    else:
                        nc.scalar.activation(
                            hw[ncc][:, 0:256], ph[:, 0:256], AF.Copy, scale=w
                        )
                    nc.vector.tensor_copy(hw[ncc][:, 256:257], w)

            # ---- phase 2: out_block = adj_f @ hw, accumulated over j ----
            with tc.tile_pool(name="acc", bufs=1, space="PSUM") as acc_pool:
                acc = [
                    acc_pool.tile([128, W_FREE], dt.float32, name=f"acc{rc}")
                    for rc in range(NR)
                ]
                for jt in range(NJ // JB):
                    at = adj_pool.tile(
                        [128, JB, RB], dt.float32r, name="at", tag="at"
                    )
                    src = adjT[jt * JB * 128 : (jt + 1) * JB * 128, :].rearrange(
                        "(b p) f -> p b f", p=128
                    )
                    nc.gpsimd.dma_start(at, src)
                    for b in range(JB):
                        jc = jt * JB + b
                        for rc in range(NR):
                            nc.tensor.matmul(
                                acc[rc],
                                at[:, b, bass.ts(rc, 128)],
                                hw[jc],
                                start=(jc == 0),
                                stop=(jc == NJ - 1),
                                skip_group_check=True,
                            )

                # ---- epilogue: divide by the w-sum column, store ----
                for rc in range(NR):
                    rec = rec_pool.tile([128, 1], dt.float32, name="rec", tag="rec")
                    nc.vector.reciprocal(rec, acc[rc][:, 256:257])
                    ob = out_pool.tile([128, D], dt.float32, name="ob", tag="ob")
                    if rc % 2 == 0:
                        nc.vector.tensor_scalar_mul(ob, acc[rc][:, 0:256], rec)
                    else:
                        nc.scalar.activation(ob, acc[rc][:, 0:256], AF.Copy, scale=rec)
                    nc.sync.dma_start(out[rc * 128 : (rc + 1) * 128, :], ob)

    _legalize_multiwait(nc, max_keep=1)
    return nc


_CACHED = {}


def _prep_inputs(x, adj, W, a):
    xT = np.ascontiguousarray(x.T).astype(np.float32, copy=False)
    WTe = np.zeros((D, W_FREE), dtype=np.float32)
    WTe[:, :256] = W.T
    WTe[:, 256] = (W.T.astype(np.float64) @ a[256:].astype(np.float64)).astype(
        np.float32
    )
    in_maps = []
    for c in range(NCORES):
        adjT_c = np.ascontiguousarray(adj[c * RB : (c + 1) * RB, :].T)
        in_maps.append({"xT": xT, "WTe": WTe, "adjT": adjT_c})
    return in_maps


def _run(in_maps, **kw):
    if "nc" not in _CACHED:
        _CACHED["nc"] = _build_program()
    return run_bass_kernel_spmd(
        _CACHED["nc"], in_maps, core_ids=list(range(NCORES)), **kw
    )


def kernel(x, adj, W, a):
    in_maps = _prep_inputs(x, adj, W, a)
    res = _run(in_maps)
    return np.concatenate([r["out"] for r in res.results], axis=0)


# revision 5
# speedup vs baseline: 1.0918x; 1.0918x over previous
"""GAT layer kernel for Trainium2, sharded across 8 NeuronCores.

Math: reference computes
    h = x @ W.T;  e_ij = (h @ a1)[i] + (h @ a2)[j];  mask by adj;
    softmax over j; out = attn @ h.
Because e_i is constant along the softmax axis it cancels, so with
w_j = exp(h_j . a2):
    out[i] = sum_j adj[i,j] * w_j * h[j] / sum_j adj[i,j] * w_j
which is a single (adj_f32 @ [w*h | w]) matmul plus a row division.
a1 is mathematically irrelevant.

Sharding: rows of adj (and of the output) are split across 8 cores;
h (8 MB) is small, so every core computes the full h redundantly
(cheaper than a collective). The host passes adj row-blocks
TRANSPOSED so the contracted index j lands on the SBUF partition
axis with DMA-friendly strides.

dtype strategy: matmuls run in float32r (e8m11, round-to-nearest-even,
fp32 accumulate in PSUM) which streams at bf16 rate for free-dim >= 256.
adj 0/1 values are exact in f32r; the cast int32 -> f32r happens inline
in the SWDGE DMA, so no compute engine touches the 33 MB adjacency
block. Measured end-to-end error vs the fp32 reference is a few 1e-4
relative to output scale.
"""

import sys

import numpy as np

for _p in ("/opt/trn_rl_repo",):
    try:
        import concourse.bass  # noqa: F401

        break
    except ImportError:
        if _p not in sys.path:
            sys.path.insert(0, _p)

import concourse.bass as bass
import concourse.mybir as mybir
import concourse.tile as tile
from concourse.bass_utils import run_bass_kernel_spmd

dt = mybir.dt
AF = mybir.ActivationFunctionType

N = 8192
D = 256
NCORES = 8
RB = N // NCORES  # 1024 output rows per core
W_FREE = 260  # 256 h cols + 1 w col + 3 pad (f32r free dim must be even)
NJ = N // 128  # 64 j-chunks
NR = RB // 128  # 8 r-chunks per core
NI = D // 128  # 2 contraction chunks for h
SWDGE_CAST_ADJ = True  # int32 -> f32r inline in DMA

# ---------------------------------------------------------------------------
# walrus in this container accepts at most ONE sync-wait command on several
# instruction structs (Drain, 4-byte self-loading Matmult, ...) while the
# newer Tile scheduler emits more. Split the extras into single-wait
# EventSemaphore prefixes on the same engine (identical semantics).
_ev_counter = [0]


def _legalize_multiwait(nc, max_keep=1):
    for f in nc.m.functions:
        for bb in f.blocks:
            il = bb.instructions
            idx = 0
            while idx < len(il):
                inst = il[idx]
                si = inst.sync_info
                if si is not None and si.on_wait and len(si.on_wait) > max_keep:
                    waits = list(si.on_wait)
                    keep = waits[len(waits) - max_keep :] if max_keep else []
                    extra = waits[: len(waits) - max_keep] if max_keep else waits
                    si.on_wait = keep
                    for w in extra:
                        _ev_counter[0] += 1
                        ev = mybir.InstEventSemaphore(
                            name=f"lgw_{_ev_counter[0]}", ins=[], outs=[]
                        )
                        ev.engine = inst.engine
                        ev.sync_info = mybir.SyncInfo(on_wait=[w], on_update=[])
                        il.insert(idx, ev)
                        idx += 1
                idx += 1


# ---------------------------------------------------------------------------


def _build_program():
    nc = bass.Bass("TRN2", debug=False)

    xT = nc.dram_tensor("xT", [D, N], dt.float32, kind="ExternalInput").ap()
    WTe = nc.dram_tensor("WTe", [D, W_FREE], dt.float32, kind="ExternalInput").ap()
    adjT = nc.dram_tensor(
        "adjT", [N, RB], dt.int32, kind="ExternalInput"
    ).ap()  # adj rows of this core, transposed: [j, r]
    out = nc.dram_tensor("out", [RB, D], dt.float32, kind="ExternalOutput").ap()

    XCH = 2048  # x streamed in [128, XCH] chunks (1 MB per DMA)
    NXB = N // XCH  # 8 chunks per i-half
    JB = 2  # j-chunks per adjT DMA (1 MB per transfer)

    with tile.TileContext(nc) as tc:
        with (
            tc.tile_pool(name="xr", bufs=1) as xr_pool,
            tc.tile_pool(name="wte", bufs=1) as wte_pool,
            tc.tile_pool(name="hw", bufs=1) as hw_pool,
            tc.tile_pool(name="wcol", bufs=4) as w_pool,
            tc.tile_pool(name="adjr", bufs=6) as adj_pool,
            tc.tile_pool(name="outs", bufs=4) as out_pool,
            tc.tile_pool(name="rec", bufs=4) as rec_pool,
        ):
            # ---- load x^T and W^T_ext, casting f32 -> f32r in the DMA.
            # SWDGE drains its single queue in issue order, so these chunked
            # transfers all land ahead of the big adjacency stream and phase 1
            # can start within a few us.
            wte = []
            for ic in range(NI):
                t = wte_pool.tile([128, W_FREE], dt.float32r, name=f"wte{ic}")
                nc.gpsimd.dma_start(t, WTe[ic * 128 : (ic + 1) * 128, :])
                wte.append(t)
            xr = [[None] * NXB for _ in range(NI)]
            for b in range(NXB):
                for ic in range(NI):
                    t = xr_pool.tile([128, XCH], dt.float32r, name=f"xr{ic}_{b}")
                    nc.gpsimd.dma_start(
                        t, xT[ic * 128 : (ic + 1) * 128, b * XCH : (b + 1) * XCH]
                    )
                    xr[ic][b] = t

            hw = [
                hw_pool.tile([128, W_FREE], dt.float32r, name=f"hw{j}")
                for j in range(NJ)
            ]

            # ---- phase 1: h plus e in one matmul; build hw = [w*h | w] ----
            NCPB = XCH // 128  # n-chunks per x chunk
            with tc.tile_pool(name="ph", bufs=4, space="PSUM") as ph_pool:
                for ncc in range(NJ):
                    b, sl = ncc // NCPB, bass.ts(ncc % NCPB, 128)
                    ph = ph_pool.tile([128, W_FREE], dt.float32, name="ph", tag="ph")
                    nc.tensor.matmul(
                        ph, xr[0][b][:, sl], wte[0], start=True, stop=False
                    )
                    nc.tensor.matmul(
                        ph, xr[1][b][:, sl], wte[1], start=False, stop=True
                    )
                    w = w_pool.tile([128, 1], dt.float32, name="w", tag="w")
                    nc.scalar.activation(w, ph[:, 256:257], AF.Exp)
                    if ncc % 2 == 0:
                        nc.vector.tensor_scalar_mul(
                            hw[ncc][:, 0:256], ph[:, 0:256], w
                        )
                    else:
                        nc.scalar.activation(
                            hw[ncc][:, 0:256], ph[:, 0:256], AF.Copy, scale=w
                        )
                    nc.vector.tensor_copy(hw[ncc][:, 256:257], w)

            # ---- phase 2: out_block = adj_f @ hw, accumulated over j ----
            with tc.tile_pool(name="acc", bufs=1, space="PSUM") as acc_pool:
                acc = [
                    acc_pool.tile([128, W_FREE], dt.float32, name=f"acc{rc}")
                    for rc in range(NR)
                ]
                for jt in range(NJ // JB):
                    at = adj_pool.tile(
                        [128, JB, RB], dt.float32r, name="at", tag="at"
                    )
                    src = adjT[jt * JB * 128 : (jt + 1) * JB * 128, :].rearrange(
                        "(b p) f -> p b f", p=128
                    )
                    nc.gpsimd.dma_start(at, src)
                    for b in range(JB):
                        jc = jt * JB + b
                        for rc in range(NR):
                            nc.tensor.matmul(
                                acc[rc],
                                at[:, b, bass.ts(rc, 128)],
                                hw[jc],
                                start=(jc == 0),
                                stop=(jc == NJ - 1),
                                skip_group_check=True,
                            )

                # ---- epilogue: divide by the w-sum column, store ----
                for rc in range(NR):
                    rec = rec_pool.tile([128, 1], dt.float32, name="rec", tag="rec")
                    nc.vector.reciprocal(rec, acc[rc][:, 256:257])
                    ob = out_pool.tile([128, D], dt.float32, name="ob", tag="ob")
                    if rc % 2 == 0:
                        nc.vector.tensor_scalar_mul(ob, acc[rc][:, 0:256], rec)
                    else:
                        nc.scalar.activation(ob, acc[rc][:, 0:256], AF.Copy, scale=rec)
                    nc.sync.dma_start(out[rc * 128 : (rc + 1) * 128, :], ob)

    _legalize_multiwait(nc, max_keep=1)
    return nc


_CACHED = {}


def _prep_inputs(x, adj, W, a):
    xT = np.ascontiguousarray(x.T).astype(np.float32, copy=False)
    WTe = np.zeros((D, W_FREE), dtype=np.float32)
    WTe[:, :256] = W.T
    WTe[:, 256] = (W.T.astype(np.float64) @ a[256:].astype(np.float64)).astype(
        np.float32
    )
    in_maps = []
    for c in range(NCORES):
        adjT_c = np.ascontiguousarray(adj[c * RB : (c + 1) * RB, :].T)
        in_maps.append({"xT": xT, "WTe": WTe, "adjT": adjT_c})
    return in_maps


def _run(in_maps, **kw):
    if "nc" not in _CACHED:
        _CACHED["nc"] = _build_program()
    return run_bass_kernel_spmd(
        _CACHED["nc"], in_maps, core_ids=list(range(NCORES)), **kw
    )


def kernel(x, adj, W, a):
    in_maps = _prep_inputs(x, adj, W, a)
    res = _run(in_maps)
    return np.concatenate([r["out"] for r in res.results], axis=0)
